# revision 53
# baseline (speedup 1.0000x reference)
"""Causal self-attention (B=4, T=2048, C=1024, 16 heads) on 8 trn2 NeuronCores.

Sharding: core (b, hg) handles batch b (4) x head-group hg (2 groups of 8 heads).
Each core computes QKV projection for its batch restricted to its 8 heads,
flash-style causal attention, and the output projection restricted to its
heads' rows of w_proj -> a partial [T, C] output. Host sums the two partials
per batch (tensor-parallel unshard) and concatenates batches.

Key layout choices (all bf16 matmul inputs, fp32 PSUM accumulation):
  - x is fed pre-transposed per batch: xT [C, T], so Q^T/K^T [d, t] come
    straight out of the QKV matmuls (lhsT = w slice, rhs = xT).
  - Scores are computed TRANSPOSED: S^T[tk, q] = matmul with lhsT = K^T chunk,
    rhs = Q^T chunk. Softmax exp runs on ScalarE from PSUM.
  - A@V produces y~^T[d, q] directly with the softmax denominator riding along
    in the same stream: even heads use lhsT=[V|1] (M=65, l in psum row 64),
    odd heads use lhsT=[1|0..0|V] (M=128, l in row 0, y~ in rows 64..127).
    The denominator row is copied to SBUF (ScalarE, fp16), broadcast across
    the 64 head partitions by a K=1 PE matmul against a ones vector
    (tile-positioned so the broadcast lands on the same partitions as y~),
    inverted with reciprocal_approx_fast, and multiplied in -> y^T written
    straight into the projection-input layout. No PE transposes, no [q]-major
    y buffer.
  - y^T overwrites Q^T in place: pair p's q-chunk c of Q^T is dead as soon as
    chunk c's scores are done, and its slot/columns are exactly where the
    output projection wants y^T.
  - Causal masking: block-skipping, matmul column ranges narrowed to the valid
    q-range on diagonal slots, exp written only to the valid range of a
    dedicated diag buffer (whose sub-diagonal zeros are written once), and one
    [128,128] staircase mask multiplied into the true diagonal blocks.
  - Next pair's QKV projection matmuls are interleaved into the attention
    c-loop as PE filler while ScalarE works through the exps; pair 3
    interleaves the output-projection tail the same way.
"""

import numpy as np
import ml_dtypes

B, T, C, H, D = 4, 2048, 1024, 16, 64
P = 128
TC = T // P          # 16 t-chunks of 128
KC = C // P          # 8 contraction chunks of 128
KC2 = 4              # fp8 DoubleRow contraction chunks of 256 (128 part x 2)
NPAIR = 4            # head pairs per core (8 local heads)
SCALE = 0.125        # 1/sqrt(64)
SW = 64.0            # w_attn(qk) fp8 scale

_CACHE = {}
LAST_RESULT = None   # BassKernelResults of the most recent run (for test.py)

BF16 = ml_dtypes.bfloat16
F8 = ml_dtypes.float8_e4m3


def _build_program():
    import concourse.tile as tile
    import concourse.mybir as mybir
    from concourse import bacc

    dt = mybir.dt
    AF = mybir.ActivationFunctionType
    ALU = mybir.AluOpType
    DR = mybir.MatmulPerfMode.DoubleRow

    nc = bacc.Bacc("TRN2", target_bir_lowering=False, debug=False, num_devices=8)

    # ---- DRAM I/O ----
    xT_d = nc.dram_tensor("xT", [C, T], dt.bfloat16, kind="ExternalInput").ap()
    x8_d = nc.dram_tensor("x8", [C, T], dt.float8e4, kind="ExternalInput").ap()
    wqk8_d = nc.dram_tensor("wqk8", [C, 1024], dt.float8e4, kind="ExternalInput").ap()
    wv_d = nc.dram_tensor("wv", [C, 512], dt.bfloat16, kind="ExternalInput").ap()
    wproj_d = nc.dram_tensor("wproj", [512, C], dt.bfloat16, kind="ExternalInput").ap()
    bqk_d = nc.dram_tensor("bqk", [P, 8], dt.float32, kind="ExternalInput").ap()
    bv_d = nc.dram_tensor("bv", [P, 512], dt.bfloat16, kind="ExternalInput").ap()
    bproj_d = nc.dram_tensor("bproj", [P, C], dt.bfloat16, kind="ExternalInput").ap()
    dmask_d = nc.dram_tensor("dmask", [P, P], dt.bfloat16, kind="ExternalInput").ap()
    sel_d = nc.dram_tensor("sel", [2, P], dt.bfloat16, kind="ExternalInput").ap()
    out_d = nc.dram_tensor("out", [T, C], dt.bfloat16, kind="ExternalOutput").ap()

    with tile.TileContext(nc) as tc:
        with (
            tc.tile_pool(name="const", bufs=1) as cp,
            tc.tile_pool(name="outp", bufs=4) as op_pool,
            tc.tile_pool(name="small", bufs=4) as sp,
            tc.tile_pool(name="rinvp", bufs=2) as rp,
            tc.tile_pool(name="psqk", bufs=2, space="PSUM") as psqk_pool,
            tc.tile_pool(name="psmm", bufs=3, space="PSUM") as psmm_pool,
            tc.tile_pool(name="psl", bufs=1, space="PSUM") as psl_pool,
        ):
            # ---- static SBUF tensors ----
            xT_s = cp.tile([P, KC, T], dt.bfloat16, name="xT_s")
            x8_s = cp.tile([P, KC2, 2, T], dt.float8e4, name="x8_s")
            wqk8_s = cp.tile([P, KC2, 2, 1024], dt.float8e4, name="wqk8_s")
            wv_s = cp.tile([P, KC, 512], dt.bfloat16, name="wv_s")
            wproj_s = cp.tile([P, 4, C], dt.bfloat16, name="wproj_s")
            bqk_s = cp.tile([P, 8], dt.float32, name="bqk_s")
            sel_s = cp.tile([P, 128], dt.bfloat16, name="sel_s")  # rows 0-1 used
            bv_s = cp.tile([P, 512], dt.bfloat16, name="bv_s")
            bproj_s = cp.tile([P, C], dt.bfloat16, name="bproj_s")
            dmask_s = cp.tile([P, P], dt.bfloat16, name="dmask_s")
            ones_s = cp.tile([P, 64], dt.bfloat16, name="ones_s")
            qt_s = cp.tile([P, NPAIR, T], dt.bfloat16, name="qt_s")  # reused as y^T
            kt_s = cp.tile([P, NPAIR, T], dt.bfloat16, name="kt_s")
            ve_s = cp.tile([P, TC, NPAIR, 66], dt.bfloat16, name="ve_s")   # [V|1|pad]
            vo_s = cp.tile([P, TC, NPAIR, 128], dt.bfloat16, name="vo_s")  # [1|0|V]
            pt_s = cp.tile([P, 12, 2, 512], dt.bfloat16, name="pt_s")   # exp(S^T) off-diag
            # diag slots, DIAG-ALIGNED: slot r's exp stored shifted by -128r so
            # the staircase mask batches into one strided op per head
            ptd_s = cp.tile([P, 2, 4, 512], dt.bfloat16, name="ptd_s")

            # ---- input DMAs (weights for the first compute first, split
            # finely so the first V-proj matmuls are gated by ~256KB/queue) ----
            xT_src = xT_d.rearrange("(o p) t -> p o t", p=P)
            x8_src = x8_d.rearrange("(k two p) t -> p k two t", p=P, two=2)
            wv_src = wv_d.rearrange("(o p) m -> p o m", p=P)
            wqk8_src = wqk8_d.rearrange("(k two p) m -> p k two m", p=P, two=2)
            # pair-0 QK-proj inputs first (t4-major upfront), then the V-proj
            # inputs, then the rest in first-use order
            nc.sync.dma_start(wqk8_s[:, :, :, 0:256], wqk8_src[:, :, :, 0:256])
            nc.sync.dma_start(wqk8_s[:, :, :, 512:768], wqk8_src[:, :, :, 512:768])
            nc.sync.dma_start(x8_s[:, :, :, 0:512], x8_src[:, :, :, 0:512])
            nc.sync.dma_start(bqk_s[:], bqk_d)
            nc.sync.dma_start(x8_s[:, :, :, 512:1024], x8_src[:, :, :, 512:1024])
            nc.sync.dma_start(x8_s[:, :, :, 1024:1536], x8_src[:, :, :, 1024:1536])
            nc.sync.dma_start(x8_s[:, :, :, 1536:2048], x8_src[:, :, :, 1536:2048])
            for k2 in range(4):
                nc.sync.dma_start(wv_s[:, 2 * k2:2 * k2 + 2, :],
                                  wv_src[:, 2 * k2:2 * k2 + 2, :])
            nc.sync.dma_start(xT_s[:, :, 0:256], xT_src[:, :, 0:256])
            nc.sync.dma_start(xT_s[:, :, 256:512], xT_src[:, :, 256:512])
            nc.sync.dma_start(bv_s[:], bv_d)
            nc.sync.dma_start(xT_s[:, :, 512:768], xT_src[:, :, 512:768])
            nc.sync.dma_start(xT_s[:, :, 768:1024], xT_src[:, :, 768:1024])
            nc.sync.dma_start(wqk8_s[:, :, :, 256:512], wqk8_src[:, :, :, 256:512])
            nc.sync.dma_start(wqk8_s[:, :, :, 768:1024], wqk8_src[:, :, :, 768:1024])
            for q8 in range(4, 8):
                nc.sync.dma_start(xT_s[:, :, 256 * q8:256 * (q8 + 1)],
                                  xT_src[:, :, 256 * q8:256 * (q8 + 1)])
            nc.sync.dma_start(dmask_s[:], dmask_d)
            nc.sync.dma_start(sel_s[0:2, :], sel_d)
            nc.sync.dma_start(wproj_s[:, 0:2, :],
                              wproj_d.rearrange("(o p) m -> p o m", p=P)[:, 0:2, :])
            nc.sync.dma_start(wproj_s[:, 2:4, :],
                              wproj_d.rearrange("(o p) m -> p o m", p=P)[:, 2:4, :])
            nc.sync.dma_start(bproj_s[:], bproj_d)

            # ones/zeros scaffolding (the shifted diag buffer needs no zeroing:
            # A@V only reads the exp-written prefix of each diag slot)
            nc.vector.memset(ones_s[:], 1.0)
            nc.vector.memset(ve_s[:, :, :, 64:65], 1.0)
            nc.vector.memset(vo_s[:, :, :, 0:1], 1.0)
            nc.vector.memset(vo_s[:, :, :, 1:64], 0.0)
            # PE warmup spin during the input-DMA head: the tensor engine
            # needs ~3us of continuous work to ramp to its max p-state
            for _ in range(48):
                wu = psmm_pool.tile([P, 512], dt.float32, name="wu", tag="mm")
                nc.tensor.matmul(wu[0:64, 0:64], ones_s[:, :], ones_s[:, :],
                                 start=True, stop=True)



            # ---- helper emitters ----
            def vproj_chunk(tcx):
                """V projection chunk: v[t, ch] for all 8 heads (512 cols)."""
                psv = psmm_pool.tile([P, 512], dt.float32, name="psv", tag="mm")
                for k in range(KC):
                    nc.tensor.matmul(psv[:, :],
                                     xT_s[:, k, P * tcx:P * (tcx + 1)],
                                     wv_s[:, k, :],
                                     start=(k == 0), stop=(k == KC - 1))
                psv_h = psv[:, :].rearrange("a (h d) -> a h d", h=8)
                bv_h = bv_s[:, :].rearrange("a (h d) -> a h d", h=8)
                nc.vector.tensor_add(
                    out=ve_s[:, tcx, :, 0:64],
                    in0=psv_h[:, 0::2, :], in1=bv_h[:, 0::2, :])
                nc.vector.tensor_add(
                    out=vo_s[:, tcx, :, 64:128],
                    in0=psv_h[:, 1::2, :], in1=bv_h[:, 1::2, :])

            def qkproj_chunk(m, t4):
                """One [128 out-ch, 512 t] tile of the Q^T/K^T projection:
                fp8 DoubleRow (x8 @ wqk8), psum scaled by SW, bf16 store."""
                dst = qt_s if m < 4 else kt_s
                psq = psmm_pool.tile([P, 512], dt.float32, name="psq", tag="mm")
                for k in range(KC2):
                    nc.tensor.matmul(psq[:, :],
                                     wqk8_s[:, k, :, P * m:P * (m + 1)],
                                     x8_s[:, k, :, 512 * t4:512 * (t4 + 1)],
                                     start=(k == 0), stop=(k == KC2 - 1),
                                     perf_mode=DR)
                nc.vector.tensor_scalar(
                    out=dst[:, m % 4, 512 * t4:512 * (t4 + 1)],
                    in0=psq[:, :], scalar1=1.0 / SW, scalar2=bqk_s[:, m:m + 1],
                    op0=ALU.mult, op1=ALU.add)

            def proj_chunk(tcx):
                for co in range(2):
                    psp = psmm_pool.tile([P, 512], dt.float32, name="psp", tag="mm")
                    for cc in range(4):
                        nc.tensor.matmul(psp[:, :],
                                         qt_s[:, cc, P * tcx:P * (tcx + 1)],
                                         wproj_s[:, cc, 512 * co:512 * (co + 1)],
                                         start=(cc == 0), stop=(cc == 3))
                    ot = op_pool.tile([P, 512], dt.bfloat16, name="ot", tag="ot")
                    nc.vector.tensor_add(out=ot[:, :], in0=psp[:, :],
                                         in1=bproj_s[:, 512 * co:512 * (co + 1)])
                    nc.sync.dma_start(
                        out_d[P * tcx:P * (tcx + 1), 512 * co:512 * (co + 1)], ot[:, :])

            # ---- minimal head: pair-0 projections + first four V chunks;
            # V4..15 stream into pair-0's windows so the exp stream starts
            # ~16us in instead of ~53us ----
            for t4 in range(4):
                for m in (0, 4):
                    qkproj_chunk(m, t4)
            for tcx in range(4):
                vproj_chunk(tcx)
            VFILL = {(0, 0): [4, 5, 6, 7], (0, 1): [8, 9, 10, 11],
                     (0, 2): [12, 13, 14, 15]}

            # deferred epilogues: the PE-side broadcast + DVE divide of chunk
            # c are emitted after chunk c+1's scores so the PE queue never
            # stalls on the ScalarE denominator copies
            pending = []

            def flush_pending():
                while pending:
                    pending.pop(0)()

            for pair in range(NPAIR):
                nxt = ([(m, t4) for m in (pair + 1, 5 + pair) for t4 in range(4)]
                       if pair < NPAIR - 1 else [])
                # pair 3 walks its q-chunks DESCENDING: its c=3 epilogue (the
                # gate for the last projection chunks) lands three iterations
                # early, and the kernel tail ends on the smallest chunk
                corder = [3, 2, 1, 0] if pair == NPAIR - 1 else [0, 1, 2, 3]
                prev_c = None
                for ci, c in enumerate(corder):     # q chunk of 512
                    fs = (0, 0, 1, 4, 8)   # cumulative filler chunks per c
                    fill_q = ([("v", t) for t in VFILL.get((pair, c), [])] +
                              [("qk", m, t4) for (m, t4) in nxt[fs[ci]:fs[ci + 1]]])
                    for j in range(4 * c + 4):          # tk chunk (slot)
                        r = j - 4 * c                   # >= 0 on diagonal slots
                        q0 = P * r if r >= 0 else 0     # skip masked cols
                        psS = psqk_pool.tile([P, 1024], dt.float32, name="psS",
                                             tag="psqk")
                        for hh in (0, 1):
                            base = 64 * hh
                            nc.tensor.matmul(
                                psS[:, 512 * hh + q0:512 * (hh + 1)],
                                kt_s[base:base + 64, pair, P * j:P * (j + 1)],
                                qt_s[base:base + 64, pair,
                                     512 * c + q0:512 * (c + 1)],
                                start=True, stop=True)
                        # exp( S^T * scale ), fp32 psum -> bf16 sbuf
                        if r < 0:
                            nc.scalar.activation(pt_s[:, j, :, :], psS[:, :],
                                                 AF.Exp, scale=SCALE)
                        else:
                            # diag slot: strided ACT over both heads' valid
                            # ranges into the DIAG-ALIGNED buffer (-q0 shift)
                            psS_h = psS[:, :].rearrange("a (h q) -> a h q", h=2)
                            nc.scalar.activation(
                                ptd_s[:, :, r, 0:512 - q0], psS_h[:, :, q0:],
                                AF.Exp, scale=SCALE)
                        # the deferred epilogue, projection, and filler
                        # interleave INTO the j-loop: the scores pipeline is
                        # ACT-paced (2-deep psum recycle), so the PE idles
                        # here unless independent work sits between the slots
                        if j == 1:
                            flush_pending()
                            if pair == NPAIR - 1 and ci >= 1:
                                for qi_loc in range(4):
                                    proj_chunk(4 * prev_c + qi_loc)
                        if j >= 2 and j % 3 == 2 and fill_q:
                            item = fill_q.pop(0)
                            if item[0] == "v":
                                vproj_chunk(item[1])
                            else:
                                qkproj_chunk(item[1], item[2])
                    # staircase mask on the 4 true-diagonal (shifted) blocks,
                    # one strided op per head
                    for hh in (0, 1):
                        nc.vector.tensor_tensor(
                            out=ptd_s[:, hh, :, 0:P],
                            in0=ptd_s[:, hh, :, 0:P],
                            in1=dmask_s[:, None, :].to_broadcast((P, 4, P)),
                            op=ALU.mult)
                    prev_c = c
                    # leftover filler for this iteration
                    for item in fill_q:
                        if item[0] == "v":
                            vproj_chunk(item[1])
                        else:
                            qkproj_chunk(item[1], item[2])

                    # A@V per head with the denominator riding in the same
                    # stream: even heads lhsT=[V|1] (l in row 64), odd heads
                    # lhsT=[1|0|V] (l in row 0, y~ in rows 64..127)
                    nj = 4 * c + 4
                    psyts = []
                    for hh in (0, 1):
                        psyt = psmm_pool.tile([P, 512], dt.float32, name="psyt",
                                              tag="mm")
                        psyts.append(psyt)
                        for j in range(nj):
                            r = j - 4 * c
                            if r < 0:
                                rhs = pt_s[:, j, hh, :]
                                cl = 0
                            else:
                                # diagonal slot: only columns q >= 128r live
                                # (stored shifted by -128r)
                                rhs = ptd_s[:, hh, r, 0:512 - P * r]
                                cl = P * r
                            out = (psyt[0:65, cl:512] if hh == 0
                                   else psyt[:, cl:512])
                            lhsT = (ve_s[:, j, pair, 0:65] if hh == 0
                                    else vo_s[:, j, pair, :])
                            nc.tensor.matmul(
                                out, lhsT, rhs,
                                start=(j == 0), stop=(j == nj - 1))
                    # denominator rows -> SBUF on DVE (ScalarE is the exp
                    # bottleneck); the PE broadcast + DVE divide are deferred
                    lsb = sp.tile([P, 512], dt.bfloat16, name="lsb", tag="lsb")
                    nc.vector.tensor_copy(out=lsb[0:1, :], in_=psyts[1][0:1, :])
                    nc.vector.tensor_copy(out=lsb[64:65, :], in_=psyts[0][64:65, :])

                    def mk_epilogue(pair=pair, c=c, psyts=psyts, lsb=lsb):
                        def emit():
                            # K=1 matmuls broadcast l0 -> rows 0-63,
                            # l1 -> rows 64-127 of one psl tile
                            psl = psl_pool.tile([P, 512], dt.float32,
                                                name="psl", tag="psl")
                            nc.tensor.matmul(psl[0:64, :], ones_s[64:65, :],
                                             lsb[64:65, :], start=True,
                                             stop=True, tile_position=(64, 0))
                            nc.tensor.matmul(psl[64:128, :], ones_s[0:1, :],
                                             lsb[0:1, :], start=True,
                                             stop=True, tile_position=(0, 64))
                            # one full-partition reciprocal for both heads
                            # (reciprocal_approx_fast is broken at base
                            # partition 64)
                            rinv = rp.tile([P, 512], dt.float32, name="rinv",
                                           tag="rinv")
                            nc.vector.reciprocal_approx_fast(out=rinv[:, :],
                                                            in_=psl[:, :])
                            # y^T = y~^T / l over the dead Q^T columns
                            nc.vector.tensor_tensor(
                                out=qt_s[0:64, pair, 512 * c:512 * (c + 1)],
                                in0=psyts[0][0:64, :], in1=rinv[0:64, :],
                                op=ALU.mult)
                            nc.vector.tensor_tensor(
                                out=qt_s[64:128, pair, 512 * c:512 * (c + 1)],
                                in0=psyts[1][64:128, :], in1=rinv[64:128, :],
                                op=ALU.mult)
                        return emit

                    pending.append(mk_epilogue())

            # tail: last chunk's epilogue, then the final projection chunks
            # (pair 3 ends on c=0, so these are token chunks 0..3)
            flush_pending()
            for qi_loc in range(4):
                proj_chunk(qi_loc)

    nc.compile()
    return nc


def _prep_inputs(x, w_attn, b_attn, w_proj, b_proj):
    """Host-side shard prep: per-core input dicts (core ci = b*2 + hg)."""
    x = np.asarray(x, dtype=np.float32)
    w_attn = np.asarray(w_attn, dtype=np.float32)
    b_attn = np.asarray(b_attn, dtype=np.float32)
    w_proj = np.asarray(w_proj, dtype=np.float32)
    b_proj = np.asarray(b_proj, dtype=np.float32)

    # diagonal staircase mask [tk, q]: valid iff q >= tk
    dmask = (np.arange(P)[None, :] >= np.arange(P)[:, None]).astype(BF16)

    in_maps = []
    for b in range(B):
        xTf = np.ascontiguousarray(x[b].T)                   # [C, T] f32
        xT = xTf.astype(BF16)
        x8 = xTf.astype(F8)
        for hg in range(2):
            lo = hg * 512
            wqk = np.concatenate(
                [w_attn[:, lo:lo + 512], w_attn[:, 1024 + lo:1024 + lo + 512]],
                axis=1)                                       # [C, 1024] f32
            wqk8 = np.ascontiguousarray(wqk * SW).astype(F8)
            wv = w_attn[:, 2048 + lo:2048 + lo + 512].astype(BF16)
            wproj = w_proj[lo:lo + 512, :].astype(BF16)       # [512, C]
            bqk = np.stack(
                [b_attn[lo + P * m:lo + P * (m + 1)] for m in range(4)] +
                [b_attn[1024 + lo + P * m:1024 + lo + P * (m + 1)] for m in range(4)],
                axis=1).astype(np.float32)                    # [128, 8]
            bv = np.broadcast_to(b_attn[2048 + lo:2048 + lo + 512],
                                 (P, 512)).astype(BF16)
            bp = b_proj if hg == 0 else np.zeros_like(b_proj)
            bproj = np.broadcast_to(bp, (P, C)).astype(BF16)
            # broadcast selector: row 0 (odd head's l) -> psl rows 64..127,
            # row 1 (even head's l) -> psl rows 0..63
            sel = np.zeros((2, P), dtype=BF16)
            sel[0, 64:128] = 1.0
            sel[1, 0:64] = 1.0
            in_maps.append({
                "xT": xT, "x8": x8, "wqk8": wqk8, "wv": wv, "wproj": wproj,
                "sel": sel,
                "bqk": np.ascontiguousarray(bqk), "bv": np.ascontiguousarray(bv),
                "bproj": np.ascontiguousarray(bproj),
                "dmask": np.ascontiguousarray(dmask),
            })
    return in_maps


def kernel(x, w_attn, b_attn, w_proj, b_proj):
    global LAST_RESULT
    from concourse.bass_utils import run_bass_kernel_spmd

    if "nc" not in _CACHE:
        _CACHE["nc"] = _build_program()
    nc = _CACHE["nc"]

    in_maps = _prep_inputs(x, w_attn, b_attn, w_proj, b_proj)
    res = run_bass_kernel_spmd(nc, in_maps, core_ids=list(range(8)))
    LAST_RESULT = res

    out = np.zeros((B, T, C), dtype=np.float32)
    for b in range(B):
        out[b] = (res.results[2 * b]["out"].astype(np.float32) +
                  res.results[2 * b + 1]["out"].astype(np.float32))
    return out



# revision 55
# speedup vs baseline: 1.0447x; 1.0447x over previous
"""Causal self-attention (B=4, T=2048, C=1024, 16 heads) on 8 trn2 NeuronCores.

Sharding: core (b, hg) handles batch b (4) x head-group hg (2 groups of 8 heads).
Each core computes QKV projection for its batch restricted to its 8 heads,
flash-style causal attention, and the output projection restricted to its
heads' rows of w_proj -> a partial [T, C] output. Host sums the two partials
per batch (tensor-parallel unshard) and concatenates batches.

Key layout choices (all bf16 matmul inputs, fp32 PSUM accumulation):
  - x is fed pre-transposed per batch: xT [C, T], so Q^T/K^T [d, t] come
    straight out of the QKV matmuls (lhsT = w slice, rhs = xT).
  - Scores are computed TRANSPOSED: S^T[tk, q] = matmul with lhsT = K^T chunk,
    rhs = Q^T chunk. Softmax exp runs on ScalarE from PSUM.
  - A@V produces y~^T[d, q] directly with the softmax denominator riding along
    in the same stream: even heads use lhsT=[V|1] (M=65, l in psum row 64),
    odd heads use lhsT=[1|0..0|V] (M=128, l in row 0, y~ in rows 64..127).
    The denominator row is copied to SBUF (ScalarE, fp16), broadcast across
    the 64 head partitions by a K=1 PE matmul against a ones vector
    (tile-positioned so the broadcast lands on the same partitions as y~),
    inverted with reciprocal_approx_fast, and multiplied in -> y^T written
    straight into the projection-input layout. No PE transposes, no [q]-major
    y buffer.
  - y^T overwrites Q^T in place: pair p's q-chunk c of Q^T is dead as soon as
    chunk c's scores are done, and its slot/columns are exactly where the
    output projection wants y^T.
  - Causal masking: block-skipping, matmul column ranges narrowed to the valid
    q-range on diagonal slots, exp written only to the valid range of a
    dedicated diag buffer (whose sub-diagonal zeros are written once), and one
    [128,128] staircase mask multiplied into the true diagonal blocks.
  - Next pair's QKV projection matmuls are interleaved into the attention
    c-loop as PE filler while ScalarE works through the exps; pair 3
    interleaves the output-projection tail the same way.
"""

import numpy as np
import ml_dtypes

B, T, C, H, D = 4, 2048, 1024, 16, 64
P = 128
TC = T // P          # 16 t-chunks of 128
KC = C // P          # 8 contraction chunks of 128
KC2 = 4              # fp8 DoubleRow contraction chunks of 256 (128 part x 2)
NPAIR = 4            # head pairs per core (8 local heads)
SCALE = 0.125        # 1/sqrt(64)
SW = 64.0            # w_attn(qk) fp8 scale

_CACHE = {}
LAST_RESULT = None   # BassKernelResults of the most recent run (for test.py)

BF16 = ml_dtypes.bfloat16
F8 = ml_dtypes.float8_e4m3


def _build_program():
    import concourse.tile as tile
    import concourse.mybir as mybir
    from concourse import bacc

    dt = mybir.dt
    AF = mybir.ActivationFunctionType
    ALU = mybir.AluOpType
    DR = mybir.MatmulPerfMode.DoubleRow

    nc = bacc.Bacc("TRN2", target_bir_lowering=False, debug=False, num_devices=8)

    # ---- DRAM I/O ----
    xT_d = nc.dram_tensor("xT", [C, T], dt.bfloat16, kind="ExternalInput").ap()
    x8_d = nc.dram_tensor("x8", [C, T], dt.float8e4, kind="ExternalInput").ap()
    wqk8_d = nc.dram_tensor("wqk8", [C, 1024], dt.float8e4, kind="ExternalInput").ap()
    wv_d = nc.dram_tensor("wv", [C, 512], dt.bfloat16, kind="ExternalInput").ap()
    wproj_d = nc.dram_tensor("wproj", [512, C], dt.bfloat16, kind="ExternalInput").ap()
    bqk_d = nc.dram_tensor("bqk", [P, 8], dt.float32, kind="ExternalInput").ap()
    bv_d = nc.dram_tensor("bv", [P, 512], dt.bfloat16, kind="ExternalInput").ap()
    bproj_d = nc.dram_tensor("bproj", [P, C], dt.bfloat16, kind="ExternalInput").ap()
    dmask_d = nc.dram_tensor("dmask", [P, P], dt.bfloat16, kind="ExternalInput").ap()
    sel_d = nc.dram_tensor("sel", [2, P], dt.bfloat16, kind="ExternalInput").ap()
    out_d = nc.dram_tensor("out", [T, C], dt.bfloat16, kind="ExternalOutput").ap()

    with tile.TileContext(nc) as tc:
        with (
            tc.tile_pool(name="const", bufs=1) as cp,
            tc.tile_pool(name="outp", bufs=4) as op_pool,
            tc.tile_pool(name="small", bufs=4) as sp,
            tc.tile_pool(name="rinvp", bufs=2) as rp,
            tc.tile_pool(name="psqk", bufs=2, space="PSUM") as psqk_pool,
            tc.tile_pool(name="psmm", bufs=3, space="PSUM") as psmm_pool,
            tc.tile_pool(name="psl", bufs=1, space="PSUM") as psl_pool,
        ):
            # ---- static SBUF tensors ----
            xT_s = cp.tile([P, KC, T], dt.bfloat16, name="xT_s")
            x8_s = cp.tile([P, KC2, 2, T], dt.float8e4, name="x8_s")
            wqk8_s = cp.tile([P, KC2, 2, 1024], dt.float8e4, name="wqk8_s")
            wv_s = cp.tile([P, KC, 512], dt.bfloat16, name="wv_s")
            wproj_s = cp.tile([P, 4, C], dt.bfloat16, name="wproj_s")
            bqk_s = cp.tile([P, 8], dt.float32, name="bqk_s")
            sel_s = cp.tile([P, 128], dt.bfloat16, name="sel_s")  # rows 0-1 used
            bv_s = cp.tile([P, 512], dt.bfloat16, name="bv_s")
            bproj_s = cp.tile([P, C], dt.bfloat16, name="bproj_s")
            dmask_s = cp.tile([P, P], dt.bfloat16, name="dmask_s")
            ones_s = cp.tile([P, 64], dt.bfloat16, name="ones_s")
            qt_s = cp.tile([P, NPAIR, T], dt.bfloat16, name="qt_s")  # reused as y^T
            kt_s = cp.tile([P, NPAIR, T], dt.bfloat16, name="kt_s")
            ve_s = cp.tile([P, TC, NPAIR, 128], dt.bfloat16, name="ve_s")  # [V|1|0pad]
            vo_s = cp.tile([P, TC, NPAIR, 128], dt.bfloat16, name="vo_s")  # [1|0|V]
            pt_s = cp.tile([P, 12, 2, 512], dt.bfloat16, name="pt_s")   # exp(S^T) off-diag
            # diag slots, DIAG-ALIGNED: slot r's exp stored shifted by -128r so
            # the staircase mask batches into one strided op per head
            ptd_s = cp.tile([P, 2, 4, 512], dt.bfloat16, name="ptd_s")

            # ---- input DMAs (weights for the first compute first, split
            # finely so the first V-proj matmuls are gated by ~256KB/queue) ----
            xT_src = xT_d.rearrange("(o p) t -> p o t", p=P)
            x8_src = x8_d.rearrange("(k two p) t -> p k two t", p=P, two=2)
            wv_src = wv_d.rearrange("(o p) m -> p o m", p=P)
            wqk8_src = wqk8_d.rearrange("(k two p) m -> p k two m", p=P, two=2)
            # pair-0 QK-proj inputs first (t4-major upfront), then the V-proj
            # inputs, then the rest in first-use order
            nc.sync.dma_start(wqk8_s[:, :, :, 0:256], wqk8_src[:, :, :, 0:256])
            nc.sync.dma_start(wqk8_s[:, :, :, 512:768], wqk8_src[:, :, :, 512:768])
            nc.sync.dma_start(x8_s[:, :, :, 0:512], x8_src[:, :, :, 0:512])
            nc.sync.dma_start(bqk_s[:], bqk_d)
            nc.sync.dma_start(x8_s[:, :, :, 512:1024], x8_src[:, :, :, 512:1024])
            nc.sync.dma_start(x8_s[:, :, :, 1024:1536], x8_src[:, :, :, 1024:1536])
            nc.sync.dma_start(x8_s[:, :, :, 1536:2048], x8_src[:, :, :, 1536:2048])
            for k2 in range(4):
                nc.sync.dma_start(wv_s[:, 2 * k2:2 * k2 + 2, :],
                                  wv_src[:, 2 * k2:2 * k2 + 2, :])
            nc.sync.dma_start(xT_s[:, :, 0:256], xT_src[:, :, 0:256])
            nc.sync.dma_start(xT_s[:, :, 256:512], xT_src[:, :, 256:512])
            nc.sync.dma_start(bv_s[:], bv_d)
            nc.sync.dma_start(xT_s[:, :, 512:768], xT_src[:, :, 512:768])
            nc.sync.dma_start(xT_s[:, :, 768:1024], xT_src[:, :, 768:1024])
            nc.sync.dma_start(wqk8_s[:, :, :, 256:512], wqk8_src[:, :, :, 256:512])
            nc.sync.dma_start(wqk8_s[:, :, :, 768:1024], wqk8_src[:, :, :, 768:1024])
            for q8 in range(4, 8):
                nc.sync.dma_start(xT_s[:, :, 256 * q8:256 * (q8 + 1)],
                                  xT_src[:, :, 256 * q8:256 * (q8 + 1)])
            nc.sync.dma_start(dmask_s[:], dmask_d)
            nc.sync.dma_start(sel_s[0:2, :], sel_d)
            nc.sync.dma_start(wproj_s[:, 0:2, :],
                              wproj_d.rearrange("(o p) m -> p o m", p=P)[:, 0:2, :])
            nc.sync.dma_start(wproj_s[:, 2:4, :],
                              wproj_d.rearrange("(o p) m -> p o m", p=P)[:, 2:4, :])
            nc.sync.dma_start(bproj_s[:], bproj_d)

            # ones/zeros scaffolding (the shifted diag buffer needs no zeroing:
            # A@V only reads the exp-written prefix of each diag slot)
            nc.vector.memset(ones_s[:], 1.0)
            nc.vector.memset(ve_s[:, :, :, 64:65], 1.0)
            nc.vector.memset(ve_s[:, :, :, 65:128], 0.0)
            nc.vector.memset(vo_s[:, :, :, 0:1], 1.0)
            nc.vector.memset(vo_s[:, :, :, 1:64], 0.0)
            # PE warmup spin during the input-DMA head: the tensor engine
            # needs ~3us of continuous work to ramp to its max p-state
            for _ in range(48):
                wu = psmm_pool.tile([P, 512], dt.float32, name="wu", tag="mm")
                nc.tensor.matmul(wu[0:64, 0:64], ones_s[:, :], ones_s[:, :],
                                 start=True, stop=True)



            # ---- helper emitters ----
            def vproj_chunk(tcx):
                """V projection chunk: v[t, ch] for all 8 heads (512 cols)."""
                psv = psmm_pool.tile([P, 512], dt.float32, name="psv", tag="mm")
                for k in range(KC):
                    nc.tensor.matmul(psv[:, :],
                                     xT_s[:, k, P * tcx:P * (tcx + 1)],
                                     wv_s[:, k, :],
                                     start=(k == 0), stop=(k == KC - 1))
                psv_h = psv[:, :].rearrange("a (h d) -> a h d", h=8)
                bv_h = bv_s[:, :].rearrange("a (h d) -> a h d", h=8)
                nc.vector.tensor_add(
                    out=ve_s[:, tcx, :, 0:64],
                    in0=psv_h[:, 0::2, :], in1=bv_h[:, 0::2, :])
                nc.vector.tensor_add(
                    out=vo_s[:, tcx, :, 64:128],
                    in0=psv_h[:, 1::2, :], in1=bv_h[:, 1::2, :])

            def qkproj_chunk(m, t4):
                """One [128 out-ch, 512 t] tile of the Q^T/K^T projection:
                fp8 DoubleRow (x8 @ wqk8), psum scaled by SW, bf16 store."""
                dst = qt_s if m < 4 else kt_s
                psq = psmm_pool.tile([P, 512], dt.float32, name="psq", tag="mm")
                for k in range(KC2):
                    nc.tensor.matmul(psq[:, :],
                                     wqk8_s[:, k, :, P * m:P * (m + 1)],
                                     x8_s[:, k, :, 512 * t4:512 * (t4 + 1)],
                                     start=(k == 0), stop=(k == KC2 - 1),
                                     perf_mode=DR)
                nc.vector.tensor_scalar(
                    out=dst[:, m % 4, 512 * t4:512 * (t4 + 1)],
                    in0=psq[:, :], scalar1=1.0 / SW, scalar2=bqk_s[:, m:m + 1],
                    op0=ALU.mult, op1=ALU.add)

            def proj_chunk(tcx):
                for co in range(2):
                    psp = psmm_pool.tile([P, 512], dt.float32, name="psp", tag="mm")
                    for cc in range(4):
                        nc.tensor.matmul(psp[:, :],
                                         qt_s[:, cc, P * tcx:P * (tcx + 1)],
                                         wproj_s[:, cc, 512 * co:512 * (co + 1)],
                                         start=(cc == 0), stop=(cc == 3))
                    ot = op_pool.tile([P, 512], dt.bfloat16, name="ot", tag="ot")
                    nc.vector.tensor_add(out=ot[:, :], in0=psp[:, :],
                                         in1=bproj_s[:, 512 * co:512 * (co + 1)])
                    nc.sync.dma_start(
                        out_d[P * tcx:P * (tcx + 1), 512 * co:512 * (co + 1)], ot[:, :])

            # ---- minimal head: pair-0 projections + first four V chunks;
            # V4..15 stream into pair-0's windows so the exp stream starts
            # ~16us in instead of ~53us ----
            for t4 in range(4):
                for m in (0, 4):
                    qkproj_chunk(m, t4)
            for tcx in range(4):
                vproj_chunk(tcx)
            VFILL = {(0, 0): [4, 5, 6, 7], (0, 1): [8, 9, 10, 11],
                     (0, 2): [12, 13, 14, 15]}

            # deferred epilogues: the PE-side broadcast + DVE divide of chunk
            # c are emitted after chunk c+1's scores so the PE queue never
            # stalls on the ScalarE denominator copies
            pending = []

            def flush_pending():
                while pending:
                    pending.pop(0)()

            for pair in range(NPAIR):
                nxt = ([(m, t4) for m in (pair + 1, 5 + pair) for t4 in range(4)]
                       if pair < NPAIR - 1 else [])
                # pair 3 walks its q-chunks DESCENDING: its c=3 epilogue (the
                # gate for the last projection chunks) lands three iterations
                # early, and the kernel tail ends on the smallest chunk
                corder = [3, 2, 1, 0] if pair == NPAIR - 1 else [0, 1, 2, 3]
                prev_c = None
                for ci, c in enumerate(corder):     # q chunk of 512
                    for j in range(4 * c + 4):          # tk chunk (slot)
                        r = j - 4 * c                   # >= 0 on diagonal slots
                        q0 = P * r if r >= 0 else 0     # skip masked cols
                        psS = psqk_pool.tile([P, 1024], dt.float32, name="psS",
                                             tag="psqk")
                        for hh in (0, 1):
                            base = 64 * hh
                            nc.tensor.matmul(
                                psS[:, 512 * hh + q0:512 * (hh + 1)],
                                kt_s[base:base + 64, pair, P * j:P * (j + 1)],
                                qt_s[base:base + 64, pair,
                                     512 * c + q0:512 * (c + 1)],
                                start=True, stop=True)
                        # exp( S^T * scale ), fp32 psum -> bf16 sbuf
                        if r < 0:
                            nc.scalar.activation(pt_s[:, j, :, :], psS[:, :],
                                                 AF.Exp, scale=SCALE)
                        else:
                            # diag slot: strided ACT over both heads' valid
                            # ranges into the DIAG-ALIGNED buffer (-q0 shift)
                            psS_h = psS[:, :].rearrange("a (h q) -> a h q", h=2)
                            nc.scalar.activation(
                                ptd_s[:, :, r, 0:512 - q0], psS_h[:, :, q0:],
                                AF.Exp, scale=SCALE)
                    # staircase mask on the 4 true-diagonal (shifted) blocks,
                    # one strided op per head
                    for hh in (0, 1):
                        nc.vector.tensor_tensor(
                            out=ptd_s[:, hh, :, 0:P],
                            in0=ptd_s[:, hh, :, 0:P],
                            in1=dmask_s[:, None, :].to_broadcast((P, 4, P)),
                            op=ALU.mult)

                    # previous chunk's deferred epilogue lands here, behind
                    # this chunk's scores in the PE queue
                    flush_pending()
                    if pair == NPAIR - 1 and ci >= 1:
                        # output projection for the q-chunk whose y^T rows
                        # were just completed by the flushed epilogue
                        for qi_loc in range(4):
                            proj_chunk(4 * prev_c + qi_loc)
                    prev_c = c

                    # PE filler while ScalarE works through the exps:
                    # pair 0 absorbs the remaining V-projection chunks; every
                    # pair pre-computes the next pair's Q^T/K^T projection,
                    # weighted toward late chunks where the exp load is largest
                    for tcx in VFILL.get((pair, c), []):
                        vproj_chunk(tcx)
                    fs = (0, 0, 1, 4, 8)   # cumulative filler chunks per c
                    for (m, t4) in nxt[fs[ci]:fs[ci + 1]]:
                        qkproj_chunk(m, t4)

                    # A@V per head with the denominator riding in the same
                    # stream: even heads lhsT=[V|1] (l in row 64), odd heads
                    # lhsT=[1|0|V] (l in row 0, y~ in rows 64..127)
                    nj = 4 * c + 4
                    psyts = []
                    for hh in (0, 1):
                        psyt = psmm_pool.tile([P, 512], dt.float32, name="psyt",
                                              tag="mm")
                        psyts.append(psyt)
                        for j in range(nj):
                            r = j - 4 * c
                            if r < 0:
                                rhs = pt_s[:, j, hh, :]
                                cl = 0
                            else:
                                # diagonal slot: only columns q >= 128r live
                                # (stored shifted by -128r)
                                rhs = ptd_s[:, hh, r, 0:512 - P * r]
                                cl = P * r
                            # even head padded to M=128 so the weight
                            # load is FWL-eligible (rows 65-127 get zeros)
                            out = psyt[:, cl:512]
                            lhsT = (ve_s[:, j, pair, :] if hh == 0
                                    else vo_s[:, j, pair, :])
                            nc.tensor.matmul(
                                out, lhsT, rhs,
                                start=(j == 0), stop=(j == nj - 1))
                    # denominator rows -> SBUF on DVE (ScalarE is the exp
                    # bottleneck); the PE broadcast + DVE divide are deferred
                    lsb = sp.tile([P, 512], dt.bfloat16, name="lsb", tag="lsb")
                    nc.vector.tensor_copy(out=lsb[0:1, :], in_=psyts[1][0:1, :])
                    nc.vector.tensor_copy(out=lsb[64:65, :], in_=psyts[0][64:65, :])

                    def mk_epilogue(pair=pair, c=c, psyts=psyts, lsb=lsb):
                        def emit():
                            # K=1 matmuls broadcast l0 -> rows 0-63,
                            # l1 -> rows 64-127 of one psl tile
                            psl = psl_pool.tile([P, 512], dt.float32,
                                                name="psl", tag="psl")
                            nc.tensor.matmul(psl[0:64, :], ones_s[64:65, :],
                                             lsb[64:65, :], start=True,
                                             stop=True, tile_position=(64, 0))
                            nc.tensor.matmul(psl[64:128, :], ones_s[0:1, :],
                                             lsb[0:1, :], start=True,
                                             stop=True, tile_position=(0, 64))
                            # one full-partition reciprocal for both heads
                            # (reciprocal_approx_fast is broken at base
                            # partition 64)
                            rinv = rp.tile([P, 512], dt.float32, name="rinv",
                                           tag="rinv")
                            nc.vector.reciprocal_approx_fast(out=rinv[:, :],
                                                            in_=psl[:, :])
                            # y^T = y~^T / l over the dead Q^T columns
                            nc.vector.tensor_tensor(
                                out=qt_s[0:64, pair, 512 * c:512 * (c + 1)],
                                in0=psyts[0][0:64, :], in1=rinv[0:64, :],
                                op=ALU.mult)
                            nc.vector.tensor_tensor(
                                out=qt_s[64:128, pair, 512 * c:512 * (c + 1)],
                                in0=psyts[1][64:128, :], in1=rinv[64:128, :],
                                op=ALU.mult)
                        return emit

                    pending.append(mk_epilogue())

            # tail: last chunk's epilogue, then the final projection chunks
            # (pair 3 ends on c=0, so these are token chunks 0..3)
            flush_pending()
            for qi_loc in range(4):
                proj_chunk(qi_loc)

    nc.compile()
    return nc


def _prep_inputs(x, w_attn, b_attn, w_proj, b_proj):
    """Host-side shard prep: per-core input dicts (core ci = b*2 + hg)."""
    x = np.asarray(x, dtype=np.float32)
    w_attn = np.asarray(w_attn, dtype=np.float32)
    b_attn = np.asarray(b_attn, dtype=np.float32)
    w_proj = np.asarray(w_proj, dtype=np.float32)
    b_proj = np.asarray(b_proj, dtype=np.float32)

    # diagonal staircase mask [tk, q]: valid iff q >= tk
    dmask = (np.arange(P)[None, :] >= np.arange(P)[:, None]).astype(BF16)

    in_maps = []
    for b in range(B):
        xTf = np.ascontiguousarray(x[b].T)                   # [C, T] f32
        xT = xTf.astype(BF16)
        x8 = xTf.astype(F8)
        for hg in range(2):
            lo = hg * 512
            wqk = np.concatenate(
                [w_attn[:, lo:lo + 512], w_attn[:, 1024 + lo:1024 + lo + 512]],
                axis=1)                                       # [C, 1024] f32
            wqk8 = np.ascontiguousarray(wqk * SW).astype(F8)
            wv = w_attn[:, 2048 + lo:2048 + lo + 512].astype(BF16)
            wproj = w_proj[lo:lo + 512, :].astype(BF16)       # [512, C]
            bqk = np.stack(
                [b_attn[lo + P * m:lo + P * (m + 1)] for m in range(4)] +
                [b_attn[1024 + lo + P * m:1024 + lo + P * (m + 1)] for m in range(4)],
                axis=1).astype(np.float32)                    # [128, 8]
            bv = np.broadcast_to(b_attn[2048 + lo:2048 + lo + 512],
                                 (P, 512)).astype(BF16)
            bp = b_proj if hg == 0 else np.zeros_like(b_proj)
            bproj = np.broadcast_to(bp, (P, C)).astype(BF16)
            # broadcast selector: row 0 (odd head's l) -> psl rows 64..127,
            # row 1 (even head's l) -> psl rows 0..63
            sel = np.zeros((2, P), dtype=BF16)
            sel[0, 64:128] = 1.0
            sel[1, 0:64] = 1.0
            in_maps.append({
                "xT": xT, "x8": x8, "wqk8": wqk8, "wv": wv, "wproj": wproj,
                "sel": sel,
                "bqk": np.ascontiguousarray(bqk), "bv": np.ascontiguousarray(bv),
                "bproj": np.ascontiguousarray(bproj),
                "dmask": np.ascontiguousarray(dmask),
            })
    return in_maps


def kernel(x, w_attn, b_attn, w_proj, b_proj):
    global LAST_RESULT
    from concourse.bass_utils import run_bass_kernel_spmd

    if "nc" not in _CACHE:
        _CACHE["nc"] = _build_program()
    nc = _CACHE["nc"]

    in_maps = _prep_inputs(x, w_attn, b_attn, w_proj, b_proj)
    res = run_bass_kernel_spmd(nc, in_maps, core_ids=list(range(8)))
    LAST_RESULT = res

    out = np.zeros((B, T, C), dtype=np.float32)
    for b in range(B):
        out[b] = (res.results[2 * b]["out"].astype(np.float32) +
                  res.results[2 * b + 1]["out"].astype(np.float32))
    return out

